# revision 2
# baseline (speedup 1.0000x reference)
"""BiLSTM Trainium2 kernel v3 — Picard iteration, 8 NeuronCores, SPMD.

The serial LSTM recurrence is latency-bound on TRN2 (~2.4us/step chain of
matmul -> sigmoid -> c-update -> tanh -> mul; 256 steps = ~610us no matter
how batch is sharded).  v3 replaces the serial scan with PICARD (fixed
point) ITERATION over the whole sequence: given a guess h^k for all t,

    gates_t = xp_t + Wh @ h^k_{t-1}        (one big GEMM, all t at once)
    i,f,g,o = sigmoid/tanh(gates)          (batched ACT)
    c_t     = f_t*c_{t-1} + i_t*g_t        (hardware tensor_tensor_scan)
    h^{k+1} = o * tanh(c)                  (batched DVE)

The map is a contraction (measured rho ~ 0.36 for these weight scales):
K sweeps give max|h err| ~ 2e-2 * rho^(K-1), i.e. ~1.6e-4 at K=6 — far
below the bf16 noise floor.  Each sweep is pure throughput (~40us), so
the whole recurrence costs K*40us instead of 256*2.4us.

Layout: tokens are BATCH-MAJOR (col j = b*256 + t), so each 512-column
jtile = 2 complete batch rows x all 256 steps and the c-scan is
self-contained per jtile (32 independent 256-long scans per sweep).
States are scaled as in v2: the scan carries c' = c/2 (data1 = i*g/2 via
one scalar_tensor_tensor), tanh(c) = 2*sigmoid(4*c') - 1 (free scale=4
in ACT), h' = h/2 = (sigmoid(4c') - 0.5)*sigma_o (one STT); Wh and W_tag
are pre-scaled x2 on the host, g-gate weights a further x2 so one
sigmoid table serves everything.

Sharding: data-parallel, identical to v1 — cores 0-3 run fwd on batch
shards of 16, cores 4-7 run the same graph on time-reversed inputs with
the backward weights; each applies half of W_tag and the host sums.
"""

import json
import os
import sys
import types
import numpy as np
import ml_dtypes

for _p in ("/root/.axon_site/_ro/trn_rl_repo", "/opt/trn_rl_repo"):
    if _p not in sys.path and os.path.isdir(_p):
        sys.path.append(_p)


def _ensure_ntff_hook():
    """This image's antenv lacks axon_hooks; synthesize it so
    run_bass_kernel_spmd(trace=True) can reach the NTFF profiler."""
    try:
        import antenv.axon_hooks  # noqa: F401
        return
    except ImportError:
        pass
    try:
        import antenv
        from trn_agent_boot.trn_boot import _ntff_profile_via_ctypes
        mod = types.ModuleType("antenv.axon_hooks")
        _hook = [None]

        def set_axon_ntff_profile_hook(h):
            _hook[0] = h

        def get_axon_ntff_profile_hook():
            if _hook[0] is None:
                try:
                    _hook[0] = _ntff_profile_via_ctypes("/opt/axon/libaxon_pjrt.so")
                except Exception:
                    return None
            return _hook[0]

        mod.set_axon_ntff_profile_hook = set_axon_ntff_profile_hook
        mod.get_axon_ntff_profile_hook = get_axon_ntff_profile_hook
        sys.modules["antenv.axon_hooks"] = mod
        antenv.axon_hooks = mod
    except Exception:
        pass


_ensure_ntff_hook()

import concourse.bass as bass
import concourse.tile as tile
from concourse import mybir
from concourse.bass_utils import run_bass_kernel_spmd

BF16 = ml_dtypes.bfloat16
F32 = mybir.dt.float32
BF = mybir.dt.bfloat16
AF = mybir.ActivationFunctionType
OP = mybir.AluOpType

E, H2, TAGS = 256, 256, 20
S = 256          # sequence length
B = 64           # global batch
BC = 16          # batch rows per core
KC = 2           # contraction chunks (E = H2 = 256 -> 2 x 128)
NJT = BC // 2    # 8 jtiles, each = 2 batch rows x 256 steps = 512 tokens
KITER = int(os.environ.get("BILSTM_K", "5"))
# slot -> original gate chunk (orig gate order i,f,g,o; 2 chunks each)
# slots: [g0,g1, i0,i1, f0,f1, o0,o1]
PERM = [4, 5, 0, 1, 2, 3, 6, 7]

_CACHE = {}
LAST_RESULT = None  # test harness introspection


def _legalize_bir_waits(raw):
    """This stack's walrus rejects any instruction carrying >=2 semaphore
    waits ("Too many sync wait commands"). Split such waits onto standalone
    single-wait EventSemaphore instructions inserted just before, on the
    same engine — semantically identical (engine streams are in-order)."""
    d = json.loads(raw)
    n = 0
    for fn in d.get("functions", []):
        for bb in fn.get("blocks", []):
            out = []
            for inst in bb.get("instructions", []):
                si = inst.get("sync_info") or {}
                waits = si.get("on_wait") or []
                if len(waits) >= 2:
                    for w_ in waits[:-1]:
                        n += 1
                        out.append({
                            "debug": inst.get("debug", 0),
                            "engine": inst["engine"],
                            "ins": [], "outs": [],
                            "name": f"legw-{n}",
                            "opcode": "EventSemaphore",
                            "sync_info": {"on_update": [], "on_wait": [w_]},
                        })
                    si = dict(si)
                    si["on_wait"] = [waits[-1]]
                    inst = dict(inst)
                    inst["sync_info"] = si
                out.append(inst)
            bb["instructions"] = out
    return json.dumps(d).encode()


def _build(with_bias=True, kiter=KITER):
    nc = bass.Bass()
    # wb packs [wx(16x128) | wh(16x128) | ident(128) | wtag(2x20)] bf16 so
    # the whole weight set arrives in ONE DMA (each SW-DGE dma_start costs
    # ~680ns of Pool-queue issue time; 11 separate DMAs = ~7.5us of startup).
    WBC = 2 * 2048 + 128 + KC * TAGS
    xsT_e = nc.declare_dram_parameter("xsT", [E, BC * S], BF, isOutput=False)
    wb_e = nc.declare_dram_parameter("wb", [128, WBC], BF, isOutput=False)
    bvbt_e = nc.declare_dram_parameter("bvbt", [128, 9], F32, isOutput=False)
    out_e = nc.declare_dram_parameter("outT", [TAGS, BC * S], F32, isOutput=True)

    with tile.TileContext(nc) as tc:
        with (
            tc.tile_pool(name="big", bufs=1) as big,
            tc.tile_pool(name="ep", bufs=3) as ep,
            tc.tile_pool(name="gp", bufs=1, space="PSUM") as gp,
        ):
            # persistent SBUF (free dims are [slot/kc, b, t], batch-major)
            xs = big.tile([128, KC, BC * S], BF)
            xp = big.tile([128, 8, BC, S], BF)
            # h' history: [p, kc, b, 1+t]; col 0 = h'_{-1} = 0. In-place
            # across Picard sweeps.
            hst = big.tile([128, KC, BC, S + 1], BF)
            wb = big.tile([128, WBC], BF)
            bvbt = big.tile([128, 9], F32)
            outb = big.tile([TAGS, BC * S], F32)
            warm = big.tile([128, 8], F32)
            c05 = big.tile([128, 2, S], F32)  # constant 0.5 for the Pool ops

            def wx_sl(slot, kc):
                o = (slot * 2 + kc) * 128
                return wb[:, o:o + 128]

            def wh_sl(slot, kc):
                o = 2048 + (slot * 2 + kc) * 128
                return wb[:, o:o + 128]

            ident = wb[:, 4096:4224]

            def wt_sl(kc):
                o = 4224 + kc * TAGS
                return wb[:, o:o + TAGS]

            bv = bvbt[:, 0:8]
            bt = bvbt[0:TAGS, 8:9]

            # ---- input DMAs (3 triggers total) ----
            nc.gpsimd.dma_start(xs[:, 0, :], xsT_e[0:128, :])
            nc.gpsimd.dma_start(xs[:, 1, :], xsT_e[128:256, :])
            nc.gpsimd.dma_start(wb[:], wb_e[:])
            nc.gpsimd.dma_start(bvbt[:], bvbt_e[:])

            nc.vector.memset(hst[:, :, :, 0:1], 0.0)
            nc.gpsimd.memset(c05[:], 0.5)
            # warm the sigmoid table before the pipeline needs it
            nc.vector.memset(warm[:], 0.0)
            nc.scalar.activation(warm[:], warm[:], AF.Sigmoid)

            ncnt = [0]

            def jtile(k, j):
                """One Picard-sweep stage for jtile j (batch rows 2j, 2j+1)."""
                # sall/s2 MUST be f32: u and h' subtract 0.5 from a sigmoid
                # output, and bf16's ~2e-3-absolute step near 0.5 becomes a
                # large relative error on small tanh values (feeds back
                # through the fixed point).  bf16 also HALVES the DVE scan
                # rate here (measured 1137ns vs 672ns per 256-col scan), so
                # f32 everywhere is also the faster choice.
                sall = ep.tile([128, 8, 2, S], F32, tag="sall")
                u = ep.tile([128, KC, 2 * S], F32, tag="u")
                cs = ep.tile([128, KC, 2, S], F32, tag="cs")
                s2 = ep.tile([128, KC, 2, S], F32, tag="s2")
                for half in range(2):
                    off = 4 * half
                    ps = gp.tile([128, 4, 2, S], F32, tag=f"g{half}")
                    if k == 0:
                        # gates = xs @ Wx (+b); also materialize xp for later
                        for s4 in range(4):
                            for kc in range(KC):
                                nc.tensor.matmul(
                                    ps[:, s4, :, :],
                                    lhsT=wx_sl(off + s4, kc),
                                    rhs=xs[:, kc, 2 * j * S:(2 * j + 2) * S],
                                    start=(kc == 0), stop=(kc == KC - 1),
                                )
                        dst = xp[:, off:off + 4, 2 * j:2 * j + 2, :]
                        sdst = sall[:, off:off + 4, :, :]
                        if with_bias:
                            nc.vector.tensor_add(
                                dst, ps[:],
                                bv[:, off:off + 4].broadcast_to([128, 4, 2, S]))
                            nc.scalar.activation(sdst, dst, AF.Sigmoid)
                        else:
                            nc.scalar.activation(sdst, ps[:], AF.Sigmoid)
                            # split the psum->xp cast across DVE and ACT so
                            # the two halves copy in parallel
                            nc.vector.tensor_copy(
                                xp[:, off:off + 2, 2 * j:2 * j + 2, :],
                                ps[:, 0:2, :, :])
                            nc.scalar.copy(
                                xp[:, off + 2:off + 4, 2 * j:2 * j + 2, :],
                                ps[:, 2:4, :, :])
                    else:
                        # gates = xp + Wh' @ h'_{t-1}
                        for s4 in range(4):
                            # one per slot: a matmul's PSUM output must stay
                            # within a single 2KB bank (512 fp32)
                            nc.tensor.matmul(
                                ps[:, s4, :, :],
                                lhsT=ident,
                                rhs=xp[:, off + s4, 2 * j:2 * j + 2, :],
                                start=True, stop=False, skip_group_check=True,
                            )
                        for s4 in range(4):
                            for kc in range(KC):
                                nc.tensor.matmul(
                                    ps[:, s4, :, :],
                                    lhsT=wh_sl(off + s4, kc),
                                    rhs=hst[:, kc, 2 * j:2 * j + 2, 0:S],
                                    start=False,
                                    stop=(s4 == 3 and kc == KC - 1),
                                    skip_group_check=True,
                                )
                        nc.scalar.activation(
                            sall[:, off:off + 4, :, :], ps[:], AF.Sigmoid)
                # u = (sg2 - 0.5) * si = i*g/2   (one STT; output must be <=3D)
                nc.vector.scalar_tensor_tensor(
                    u[:], sall[:, 0:2, :, :], 0.5, sall[:, 2:4, :, :],
                    OP.subtract, OP.mult)
                # c' scan: c'_t = sf_t * c'_{t-1} + u_t  (c' = c/2)
                for kc in range(KC):
                    for b2 in range(2):
                        nc.vector.tensor_tensor_scan(
                            cs[:, kc, b2, :], sall[:, 4 + kc, b2, :],
                            u[:, kc, b2 * S:(b2 + 1) * S], 0.0, OP.mult, OP.add)
                # s2 = sigmoid(4*c') = sigmoid(2c)
                nc.scalar.activation(s2[:], cs[:], AF.Sigmoid, scale=4.0)
                # h' = (s2 - 0.5) * so = h/2   (STT output must be <=3D).
                # Keep these on DVE: offloading one to Pool (its TTs run
                # ~1.4us each) delays hst, which gates the NEXT sweep's GEMM.
                for kc in range(KC):
                    nc.vector.scalar_tensor_tensor(
                        hst[:, kc, 2 * j:2 * j + 2, 1:S + 1], s2[:, kc, :, :],
                        0.5, sall[:, 6 + kc, :, :], OP.subtract, OP.mult)

            for k in range(kiter):
                for j in range(NJT):
                    jtile(k, j)

            # ---- tag projection: chunks of 2 batch rows (512 cols) ----
            for j in range(NJT):
                pt = gp.tile([128, 4, 2, S], F32, tag="g0")
                for kc in range(KC):
                    nc.tensor.matmul(
                        pt[0:TAGS, 0, :, :],
                        lhsT=wt_sl(kc),
                        rhs=hst[:, kc, 2 * j:2 * j + 2, 1:S + 1],
                        start=(kc == 0), stop=(kc == KC - 1),
                    )
                for b2 in range(2):
                    dst = outb[:, (2 * j + b2) * S:(2 * j + b2 + 1) * S]
                    src = pt[0:TAGS, 0, b2, :]
                    if with_bias:
                        nc.vector.tensor_add(
                            dst, src, bt[:, 0:1].broadcast_to([TAGS, S]))
                    elif (j + b2) % 2 == 0:
                        nc.vector.tensor_copy(dst, src)
                    else:
                        nc.scalar.copy(dst, src)
            for h in range(2):
                HW_ = BC * S // 2
                nc.gpsimd.dma_start(out_e[:, h * HW_:(h + 1) * HW_],
                                    outb[:, h * HW_:(h + 1) * HW_])
    return nc


def _prep_w(Wmat, hscale):
    """[256, 1024] -> [128 part, slot 8, kc 2, m 128] bf16, slot-permuted.
    g-gate slots (0,1) scaled x2 (tanh via sigmoid); everything scaled by
    hscale (2 for Wh, which consumes h' = h/2)."""
    t = Wmat.reshape(KC, 128, 8, 128)[:, :, PERM, :].astype(np.float32).copy()
    t[:, :, 0:2, :] *= 2.0
    t *= hscale
    return np.ascontiguousarray(t.transpose(1, 2, 0, 3)).astype(BF16)


def _prep_b(b):
    """[1024] -> [128, 8] f32, slot-permuted per-partition bias (g x2)."""
    b8 = b.reshape(8, 128)[PERM, :].astype(np.float32).copy()
    b8[0:2, :] *= 2.0
    return np.ascontiguousarray(b8.T)


def kernel(x, emb, Wx_f, Wh_f, b_f, Wx_b, Wh_b, b_b, W_tag, b_tag):
    x = np.asarray(x)
    emb = np.asarray(emb, np.float32)
    Wx_f, Wh_f, b_f = (np.asarray(a, np.float32) for a in (Wx_f, Wh_f, b_f))
    Wx_b, Wh_b, b_b = (np.asarray(a, np.float32) for a in (Wx_b, Wh_b, b_b))
    W_tag = np.asarray(W_tag, np.float32)
    b_tag = np.asarray(b_tag, np.float32)

    with_bias = bool(b_f.any() or b_b.any() or b_tag.any())
    key = ("nc3", with_bias, KITER)
    if key not in _CACHE:
        nc = _build(with_bias=with_bias, kiter=KITER)
        legalized = _legalize_bir_waits(nc.to_json_bytes())
        nc.to_json_bytes = lambda: legalized  # shadow: feed legalized BIR to compile
        _CACHE[key] = nc
    nc = _CACHE[key]

    embeds = emb[x]  # [B, S, E] f32
    in_maps = []
    for core in range(8):
        fwd = core < 4
        c = core % 4
        eb = embeds[c * BC:(c + 1) * BC]  # [BC, S, E]
        if not fwd:
            eb = eb[:, ::-1, :]
        # batch-major tokens: col j = b*S + t
        xsT = np.ascontiguousarray(
            eb.transpose(2, 0, 1).reshape(E, BC * S)).astype(BF16)
        Wx, Wh, bb = (Wx_f, Wh_f, b_f) if fwd else (Wx_b, Wh_b, b_b)
        wth = (W_tag[:H2] if fwd else W_tag[H2:]) * 2.0  # x2: h' = h/2
        wt_d = np.ascontiguousarray(
            wth.reshape(KC, 128, TAGS).transpose(1, 0, 2)).astype(BF16)
        bt_d = (b_tag if fwd else np.zeros_like(b_tag)).astype(np.float32)
        # pack [wx | wh | ident | wtag] into one bf16 blob (single DMA)
        wb = np.concatenate([
            _prep_w(Wx, 1.0).reshape(128, 2048),
            _prep_w(Wh, 2.0).reshape(128, 2048),
            np.eye(128, dtype=BF16),
            wt_d.reshape(128, KC * TAGS),
        ], axis=1)
        bvbt = np.zeros((128, 9), np.float32)
        bvbt[:, 0:8] = _prep_b(bb)
        bvbt[0:TAGS, 8] = bt_d
        in_maps.append({
            "xsT": xsT,
            "wb": np.ascontiguousarray(wb),
            "bvbt": bvbt,
        })

    trace = bool(os.environ.get("BILSTM_TRACE"))
    global LAST_RESULT
    kw = {}
    if trace:
        import shutil
        kw["tmpdir"] = os.environ.get("BILSTM_TRACE_DIR", "/tmp/bilstm_trace")
        shutil.rmtree(kw["tmpdir"], ignore_errors=True)
        os.makedirs(kw["tmpdir"], exist_ok=True)
    res = run_bass_kernel_spmd(nc, in_maps, core_ids=list(range(8)), trace=trace, **kw)
    LAST_RESULT = res

    outs = [np.asarray(res.results[i]["outT"], np.float32).reshape(TAGS, BC, S)
            for i in range(8)]
    out = np.empty((B, S, TAGS), np.float32)
    for c in range(4):
        tot = outs[c] + outs[c + 4][:, :, ::-1]
        out[c * BC:(c + 1) * BC] = tot.transpose(1, 2, 0)
    return out


# revision 3
# speedup vs baseline: 1.0081x; 1.0081x over previous
"""BiLSTM Trainium2 kernel v3 — Picard iteration, 8 NeuronCores, SPMD.

The serial LSTM recurrence is latency-bound on TRN2 (~2.4us/step chain of
matmul -> sigmoid -> c-update -> tanh -> mul; 256 steps = ~610us no matter
how batch is sharded).  v3 replaces the serial scan with PICARD (fixed
point) ITERATION over the whole sequence: given a guess h^k for all t,

    gates_t = xp_t + Wh @ h^k_{t-1}        (one big GEMM, all t at once)
    i,f,g,o = sigmoid/tanh(gates)          (batched ACT)
    c_t     = f_t*c_{t-1} + i_t*g_t        (hardware tensor_tensor_scan)
    h^{k+1} = o * tanh(c)                  (batched DVE)

The map is a contraction (measured rho ~ 0.36 for these weight scales):
K sweeps give max|h err| ~ 2e-2 * rho^(K-1), i.e. ~1.6e-4 at K=6 — far
below the bf16 noise floor.  Each sweep is pure throughput (~40us), so
the whole recurrence costs K*40us instead of 256*2.4us.

Layout: tokens are BATCH-MAJOR (col j = b*256 + t), so each 512-column
jtile = 2 complete batch rows x all 256 steps and the c-scan is
self-contained per jtile (32 independent 256-long scans per sweep).
States are scaled as in v2: the scan carries c' = c/2 (data1 = i*g/2 via
one scalar_tensor_tensor), tanh(c) = 2*sigmoid(4*c') - 1 (free scale=4
in ACT), h' = h/2 = (sigmoid(4c') - 0.5)*sigma_o (one STT); Wh and W_tag
are pre-scaled x2 on the host, g-gate weights a further x2 so one
sigmoid table serves everything.

Sharding: data-parallel, identical to v1 — cores 0-3 run fwd on batch
shards of 16, cores 4-7 run the same graph on time-reversed inputs with
the backward weights; each applies half of W_tag and the host sums.
"""

import json
import os
import sys
import types
import numpy as np
import ml_dtypes

for _p in ("/root/.axon_site/_ro/trn_rl_repo", "/opt/trn_rl_repo"):
    if _p not in sys.path and os.path.isdir(_p):
        sys.path.append(_p)


def _ensure_ntff_hook():
    """This image's antenv lacks axon_hooks; synthesize it so
    run_bass_kernel_spmd(trace=True) can reach the NTFF profiler."""
    try:
        import antenv.axon_hooks  # noqa: F401
        return
    except ImportError:
        pass
    try:
        import antenv
        from trn_agent_boot.trn_boot import _ntff_profile_via_ctypes
        mod = types.ModuleType("antenv.axon_hooks")
        _hook = [None]

        def set_axon_ntff_profile_hook(h):
            _hook[0] = h

        def get_axon_ntff_profile_hook():
            if _hook[0] is None:
                try:
                    _hook[0] = _ntff_profile_via_ctypes("/opt/axon/libaxon_pjrt.so")
                except Exception:
                    return None
            return _hook[0]

        mod.set_axon_ntff_profile_hook = set_axon_ntff_profile_hook
        mod.get_axon_ntff_profile_hook = get_axon_ntff_profile_hook
        sys.modules["antenv.axon_hooks"] = mod
        antenv.axon_hooks = mod
    except Exception:
        pass


_ensure_ntff_hook()

import concourse.bass as bass
import concourse.tile as tile
from concourse import mybir
from concourse.bass_utils import run_bass_kernel_spmd

BF16 = ml_dtypes.bfloat16
F32 = mybir.dt.float32
BF = mybir.dt.bfloat16
AF = mybir.ActivationFunctionType
OP = mybir.AluOpType

E, H2, TAGS = 256, 256, 20
S = 256          # sequence length
B = 64           # global batch
BC = 16          # batch rows per core
KC = 2           # contraction chunks (E = H2 = 256 -> 2 x 128)
NJT = BC // 2    # 8 jtiles, each = 2 batch rows x 256 steps = 512 tokens
KITER = int(os.environ.get("BILSTM_K", "5"))
# slot -> original gate chunk (orig gate order i,f,g,o; 2 chunks each)
# slots: [g0,g1, i0,i1, f0,f1, o0,o1]
PERM = [4, 5, 0, 1, 2, 3, 6, 7]

_CACHE = {}
LAST_RESULT = None  # test harness introspection


def _legalize_bir_waits(raw):
    """This stack's walrus rejects any instruction carrying >=2 semaphore
    waits ("Too many sync wait commands"). Split such waits onto standalone
    single-wait EventSemaphore instructions inserted just before, on the
    same engine — semantically identical (engine streams are in-order)."""
    d = json.loads(raw)
    n = 0
    for fn in d.get("functions", []):
        for bb in fn.get("blocks", []):
            out = []
            for inst in bb.get("instructions", []):
                si = inst.get("sync_info") or {}
                waits = si.get("on_wait") or []
                if len(waits) >= 2:
                    for w_ in waits[:-1]:
                        n += 1
                        out.append({
                            "debug": inst.get("debug", 0),
                            "engine": inst["engine"],
                            "ins": [], "outs": [],
                            "name": f"legw-{n}",
                            "opcode": "EventSemaphore",
                            "sync_info": {"on_update": [], "on_wait": [w_]},
                        })
                    si = dict(si)
                    si["on_wait"] = [waits[-1]]
                    inst = dict(inst)
                    inst["sync_info"] = si
                out.append(inst)
            bb["instructions"] = out
    return json.dumps(d).encode()


def _build(with_bias=True, kiter=KITER):
    nc = bass.Bass()
    # wb packs [wx(16x128) | wh(16x128) | ident(128) | wtag(2x20)] bf16 so
    # the whole weight set arrives in ONE DMA (each SW-DGE dma_start costs
    # ~680ns of Pool-queue issue time; 11 separate DMAs = ~7.5us of startup).
    WBC = 2 * 2048 + 128 + KC * TAGS
    xsT_e = nc.declare_dram_parameter("xsT", [E, BC * S], BF, isOutput=False)
    wb_e = nc.declare_dram_parameter("wb", [128, WBC], BF, isOutput=False)
    bvbt_e = nc.declare_dram_parameter("bvbt", [128, 9], F32, isOutput=False)
    out_e = nc.declare_dram_parameter("outT", [TAGS, BC * S], F32, isOutput=True)

    with tile.TileContext(nc) as tc:
        with (
            tc.tile_pool(name="big", bufs=1) as big,
            tc.tile_pool(name="ep", bufs=3) as ep,
            tc.tile_pool(name="gp", bufs=1, space="PSUM") as gp,
        ):
            # persistent SBUF (free dims are [slot/kc, b, t], batch-major)
            xs = big.tile([128, KC, BC * S], BF)
            xp = big.tile([128, 8, BC, S], BF)
            # h' history: [p, kc, b, 1+t]; col 0 = h'_{-1} = 0. In-place
            # across Picard sweeps.
            hst = big.tile([128, KC, BC, S + 1], BF)
            wb = big.tile([128, WBC], BF)
            bvbt = big.tile([128, 9], F32)
            outb = big.tile([TAGS, BC * S], F32)
            warm = big.tile([128, 8], F32)
            c05 = big.tile([128, 2, S], F32)  # constant 0.5 for the Pool ops

            def wx_sl(slot, kc):
                o = (slot * 2 + kc) * 128
                return wb[:, o:o + 128]

            def wh_sl(slot, kc):
                o = 2048 + (slot * 2 + kc) * 128
                return wb[:, o:o + 128]

            ident = wb[:, 4096:4224]

            def wt_sl(kc):
                o = 4224 + kc * TAGS
                return wb[:, o:o + TAGS]

            bv = bvbt[:, 0:8]
            bt = bvbt[0:TAGS, 8:9]

            # ---- input DMAs (3 triggers total) ----
            nc.gpsimd.dma_start(xs[:, 0, :], xsT_e[0:128, :])
            nc.gpsimd.dma_start(xs[:, 1, :], xsT_e[128:256, :])
            nc.gpsimd.dma_start(wb[:], wb_e[:])
            nc.gpsimd.dma_start(bvbt[:], bvbt_e[:])

            nc.vector.memset(hst[:, :, :, 0:1], 0.0)
            nc.gpsimd.memset(c05[:], 0.5)
            # warm the sigmoid table before the pipeline needs it
            nc.vector.memset(warm[:], 0.0)
            nc.scalar.activation(warm[:], warm[:], AF.Sigmoid)

            ncnt = [0]

            def jtile(k, j):
                """One Picard-sweep stage for jtile j (batch rows 2j, 2j+1)."""
                # sall/s2 MUST be f32: u and h' subtract 0.5 from a sigmoid
                # output, and bf16's ~2e-3-absolute step near 0.5 becomes a
                # large relative error on small tanh values (feeds back
                # through the fixed point).  bf16 also HALVES the DVE scan
                # rate here (measured 1137ns vs 672ns per 256-col scan), so
                # f32 everywhere is also the faster choice.
                sall = ep.tile([128, 8, 2, S], F32, tag="sall")
                u = ep.tile([128, KC, 2 * S], F32, tag="u")
                cs = ep.tile([128, KC, 2, S], F32, tag="cs")
                for half in range(2):
                    off = 4 * half
                    ps = gp.tile([128, 4, 2, S], F32, tag=f"g{half}")
                    if k == 0:
                        # gates = xs @ Wx (+b); also materialize xp for later
                        for s4 in range(4):
                            for kc in range(KC):
                                nc.tensor.matmul(
                                    ps[:, s4, :, :],
                                    lhsT=wx_sl(off + s4, kc),
                                    rhs=xs[:, kc, 2 * j * S:(2 * j + 2) * S],
                                    start=(kc == 0), stop=(kc == KC - 1),
                                )
                        dst = xp[:, off:off + 4, 2 * j:2 * j + 2, :]
                        sdst = sall[:, off:off + 4, :, :]
                        if with_bias:
                            nc.vector.tensor_add(
                                dst, ps[:],
                                bv[:, off:off + 4].broadcast_to([128, 4, 2, S]))
                            nc.scalar.activation(sdst, dst, AF.Sigmoid)
                        else:
                            nc.scalar.activation(sdst, ps[:], AF.Sigmoid)
                            # split the psum->xp cast across DVE and ACT so
                            # the two halves copy in parallel
                            nc.vector.tensor_copy(
                                xp[:, off:off + 2, 2 * j:2 * j + 2, :],
                                ps[:, 0:2, :, :])
                            nc.scalar.copy(
                                xp[:, off + 2:off + 4, 2 * j:2 * j + 2, :],
                                ps[:, 2:4, :, :])
                    else:
                        # gates = xp + Wh' @ h'_{t-1}
                        for s4 in range(4):
                            # one per slot: a matmul's PSUM output must stay
                            # within a single 2KB bank (512 fp32)
                            nc.tensor.matmul(
                                ps[:, s4, :, :],
                                lhsT=ident,
                                rhs=xp[:, off + s4, 2 * j:2 * j + 2, :],
                                start=True, stop=False, skip_group_check=True,
                            )
                        for s4 in range(4):
                            for kc in range(KC):
                                nc.tensor.matmul(
                                    ps[:, s4, :, :],
                                    lhsT=wh_sl(off + s4, kc),
                                    rhs=hst[:, kc, 2 * j:2 * j + 2, 0:S],
                                    start=False,
                                    stop=(s4 == 3 and kc == KC - 1),
                                    skip_group_check=True,
                                )
                        nc.scalar.activation(
                            sall[:, off:off + 4, :, :], ps[:], AF.Sigmoid)
                # u = (sg2 - 0.5) * si = i*g/2   (one STT; output must be <=3D)
                nc.vector.scalar_tensor_tensor(
                    u[:], sall[:, 0:2, :, :], 0.5, sall[:, 2:4, :, :],
                    OP.subtract, OP.mult)
                # c' scan: c'_t = sf_t * c'_{t-1} + u_t  (c' = c/2)
                for kc in range(KC):
                    for b2 in range(2):
                        nc.vector.tensor_tensor_scan(
                            cs[:, kc, b2, :], sall[:, 4 + kc, b2, :],
                            u[:, kc, b2 * S:(b2 + 1) * S], 0.0, OP.mult, OP.add)
                return (j, sall, cs)

            def tail(pend):
                """Deferred s2/h' for a finished jtile — emitted one jtile
                later so the ACT queue never stalls on the DVE scans."""
                j, sall, cs = pend
                s2 = ep.tile([128, KC, 2, S], F32, tag="s2")
                # s2 = sigmoid(4*c') = sigmoid(2c)
                nc.scalar.activation(s2[:], cs[:], AF.Sigmoid, scale=4.0)
                # h' = (s2 - 0.5) * so = h/2   (STT output must be <=3D).
                # Keep these on DVE: offloading one to Pool (its TTs run
                # ~1.4us each) delays hst, which gates the NEXT sweep's GEMM.
                for kc in range(KC):
                    nc.vector.scalar_tensor_tensor(
                        hst[:, kc, 2 * j:2 * j + 2, 1:S + 1], s2[:, kc, :, :],
                        0.5, sall[:, 6 + kc, :, :], OP.subtract, OP.mult)

            pend = None
            for k in range(kiter):
                for j in range(NJT):
                    pend_new = jtile(k, j)
                    if pend is not None:
                        tail(pend)
                    pend = pend_new
            tail(pend)

            # ---- tag projection: chunks of 2 batch rows (512 cols) ----
            for j in range(NJT):
                pt = gp.tile([128, 4, 2, S], F32, tag="g0")
                for kc in range(KC):
                    nc.tensor.matmul(
                        pt[0:TAGS, 0, :, :],
                        lhsT=wt_sl(kc),
                        rhs=hst[:, kc, 2 * j:2 * j + 2, 1:S + 1],
                        start=(kc == 0), stop=(kc == KC - 1),
                    )
                for b2 in range(2):
                    dst = outb[:, (2 * j + b2) * S:(2 * j + b2 + 1) * S]
                    src = pt[0:TAGS, 0, b2, :]
                    if with_bias:
                        nc.vector.tensor_add(
                            dst, src, bt[:, 0:1].broadcast_to([TAGS, S]))
                    elif (j + b2) % 2 == 0:
                        nc.vector.tensor_copy(dst, src)
                    else:
                        nc.scalar.copy(dst, src)
            for h in range(2):
                HW_ = BC * S // 2
                nc.gpsimd.dma_start(out_e[:, h * HW_:(h + 1) * HW_],
                                    outb[:, h * HW_:(h + 1) * HW_])
    return nc


def _prep_w(Wmat, hscale):
    """[256, 1024] -> [128 part, slot 8, kc 2, m 128] bf16, slot-permuted.
    g-gate slots (0,1) scaled x2 (tanh via sigmoid); everything scaled by
    hscale (2 for Wh, which consumes h' = h/2)."""
    t = Wmat.reshape(KC, 128, 8, 128)[:, :, PERM, :].astype(np.float32).copy()
    t[:, :, 0:2, :] *= 2.0
    t *= hscale
    return np.ascontiguousarray(t.transpose(1, 2, 0, 3)).astype(BF16)


def _prep_b(b):
    """[1024] -> [128, 8] f32, slot-permuted per-partition bias (g x2)."""
    b8 = b.reshape(8, 128)[PERM, :].astype(np.float32).copy()
    b8[0:2, :] *= 2.0
    return np.ascontiguousarray(b8.T)


def kernel(x, emb, Wx_f, Wh_f, b_f, Wx_b, Wh_b, b_b, W_tag, b_tag):
    x = np.asarray(x)
    emb = np.asarray(emb, np.float32)
    Wx_f, Wh_f, b_f = (np.asarray(a, np.float32) for a in (Wx_f, Wh_f, b_f))
    Wx_b, Wh_b, b_b = (np.asarray(a, np.float32) for a in (Wx_b, Wh_b, b_b))
    W_tag = np.asarray(W_tag, np.float32)
    b_tag = np.asarray(b_tag, np.float32)

    with_bias = bool(b_f.any() or b_b.any() or b_tag.any())
    key = ("nc3", with_bias, KITER)
    if key not in _CACHE:
        nc = _build(with_bias=with_bias, kiter=KITER)
        legalized = _legalize_bir_waits(nc.to_json_bytes())
        nc.to_json_bytes = lambda: legalized  # shadow: feed legalized BIR to compile
        _CACHE[key] = nc
    nc = _CACHE[key]

    embeds = emb[x]  # [B, S, E] f32
    in_maps = []
    for core in range(8):
        fwd = core < 4
        c = core % 4
        eb = embeds[c * BC:(c + 1) * BC]  # [BC, S, E]
        if not fwd:
            eb = eb[:, ::-1, :]
        # batch-major tokens: col j = b*S + t
        xsT = np.ascontiguousarray(
            eb.transpose(2, 0, 1).reshape(E, BC * S)).astype(BF16)
        Wx, Wh, bb = (Wx_f, Wh_f, b_f) if fwd else (Wx_b, Wh_b, b_b)
        wth = (W_tag[:H2] if fwd else W_tag[H2:]) * 2.0  # x2: h' = h/2
        wt_d = np.ascontiguousarray(
            wth.reshape(KC, 128, TAGS).transpose(1, 0, 2)).astype(BF16)
        bt_d = (b_tag if fwd else np.zeros_like(b_tag)).astype(np.float32)
        # pack [wx | wh | ident | wtag] into one bf16 blob (single DMA)
        wb = np.concatenate([
            _prep_w(Wx, 1.0).reshape(128, 2048),
            _prep_w(Wh, 2.0).reshape(128, 2048),
            np.eye(128, dtype=BF16),
            wt_d.reshape(128, KC * TAGS),
        ], axis=1)
        bvbt = np.zeros((128, 9), np.float32)
        bvbt[:, 0:8] = _prep_b(bb)
        bvbt[0:TAGS, 8] = bt_d
        in_maps.append({
            "xsT": xsT,
            "wb": np.ascontiguousarray(wb),
            "bvbt": bvbt,
        })

    trace = bool(os.environ.get("BILSTM_TRACE"))
    global LAST_RESULT
    kw = {}
    if trace:
        import shutil
        kw["tmpdir"] = os.environ.get("BILSTM_TRACE_DIR", "/tmp/bilstm_trace")
        shutil.rmtree(kw["tmpdir"], ignore_errors=True)
        os.makedirs(kw["tmpdir"], exist_ok=True)
    res = run_bass_kernel_spmd(nc, in_maps, core_ids=list(range(8)), trace=trace, **kw)
    LAST_RESULT = res

    outs = [np.asarray(res.results[i]["outT"], np.float32).reshape(TAGS, BC, S)
            for i in range(8)]
    out = np.empty((B, S, TAGS), np.float32)
    for c in range(4):
        tot = outs[c] + outs[c + 4][:, :, ::-1]
        out[c * BC:(c + 1) * BC] = tot.transpose(1, 2, 0)
    return out


# revision 4
# speedup vs baseline: 1.0083x; 1.0003x over previous
"""BiLSTM Trainium2 kernel v3 — Picard iteration, 8 NeuronCores, SPMD.

The serial LSTM recurrence is latency-bound on TRN2 (~2.4us/step chain of
matmul -> sigmoid -> c-update -> tanh -> mul; 256 steps = ~610us no matter
how batch is sharded).  v3 replaces the serial scan with PICARD (fixed
point) ITERATION over the whole sequence: given a guess h^k for all t,

    gates_t = xp_t + Wh @ h^k_{t-1}        (one big GEMM, all t at once)
    i,f,g,o = sigmoid/tanh(gates)          (batched ACT)
    c_t     = f_t*c_{t-1} + i_t*g_t        (hardware tensor_tensor_scan)
    h^{k+1} = o * tanh(c)                  (batched DVE)

The map is a contraction (measured rho ~ 0.36 for these weight scales):
K sweeps give max|h err| ~ 2e-2 * rho^(K-1), i.e. ~1.6e-4 at K=6 — far
below the bf16 noise floor.  Each sweep is pure throughput (~40us), so
the whole recurrence costs K*40us instead of 256*2.4us.

Layout: tokens are BATCH-MAJOR (col j = b*256 + t), so each 512-column
jtile = 2 complete batch rows x all 256 steps and the c-scan is
self-contained per jtile (32 independent 256-long scans per sweep).
States are scaled as in v2: the scan carries c' = c/2 (data1 = i*g/2 via
one scalar_tensor_tensor), tanh(c) = 2*sigmoid(4*c') - 1 (free scale=4
in ACT), h' = h/2 = (sigmoid(4c') - 0.5)*sigma_o (one STT); Wh and W_tag
are pre-scaled x2 on the host, g-gate weights a further x2 so one
sigmoid table serves everything.

Sharding: data-parallel, identical to v1 — cores 0-3 run fwd on batch
shards of 16, cores 4-7 run the same graph on time-reversed inputs with
the backward weights; each applies half of W_tag and the host sums.
"""

import json
import os
import sys
import types
import numpy as np
import ml_dtypes

for _p in ("/root/.axon_site/_ro/trn_rl_repo", "/opt/trn_rl_repo"):
    if _p not in sys.path and os.path.isdir(_p):
        sys.path.append(_p)


def _ensure_ntff_hook():
    """This image's antenv lacks axon_hooks; synthesize it so
    run_bass_kernel_spmd(trace=True) can reach the NTFF profiler."""
    try:
        import antenv.axon_hooks  # noqa: F401
        return
    except ImportError:
        pass
    try:
        import antenv
        from trn_agent_boot.trn_boot import _ntff_profile_via_ctypes
        mod = types.ModuleType("antenv.axon_hooks")
        _hook = [None]

        def set_axon_ntff_profile_hook(h):
            _hook[0] = h

        def get_axon_ntff_profile_hook():
            if _hook[0] is None:
                try:
                    _hook[0] = _ntff_profile_via_ctypes("/opt/axon/libaxon_pjrt.so")
                except Exception:
                    return None
            return _hook[0]

        mod.set_axon_ntff_profile_hook = set_axon_ntff_profile_hook
        mod.get_axon_ntff_profile_hook = get_axon_ntff_profile_hook
        sys.modules["antenv.axon_hooks"] = mod
        antenv.axon_hooks = mod
    except Exception:
        pass


_ensure_ntff_hook()

import concourse.bass as bass
import concourse.tile as tile
from concourse import mybir
from concourse.bass_utils import run_bass_kernel_spmd

BF16 = ml_dtypes.bfloat16
F32 = mybir.dt.float32
BF = mybir.dt.bfloat16
AF = mybir.ActivationFunctionType
OP = mybir.AluOpType

E, H2, TAGS = 256, 256, 20
S = 256          # sequence length
B = 64           # global batch
BC = 16          # batch rows per core
KC = 2           # contraction chunks (E = H2 = 256 -> 2 x 128)
NJT = BC // 2    # 8 jtiles, each = 2 batch rows x 256 steps = 512 tokens
KITER = int(os.environ.get("BILSTM_K", "5"))
# slot -> original gate chunk (orig gate order i,f,g,o; 2 chunks each)
# slots: [g0,g1, i0,i1, f0,f1, o0,o1]
PERM = [4, 5, 0, 1, 2, 3, 6, 7]

_CACHE = {}
LAST_RESULT = None  # test harness introspection


def _legalize_bir_waits(raw):
    """This stack's walrus rejects any instruction carrying >=2 semaphore
    waits ("Too many sync wait commands"). Split such waits onto standalone
    single-wait EventSemaphore instructions inserted just before, on the
    same engine — semantically identical (engine streams are in-order)."""
    d = json.loads(raw)
    n = 0
    for fn in d.get("functions", []):
        for bb in fn.get("blocks", []):
            out = []
            for inst in bb.get("instructions", []):
                si = inst.get("sync_info") or {}
                waits = si.get("on_wait") or []
                if len(waits) >= 2:
                    for w_ in waits[:-1]:
                        n += 1
                        out.append({
                            "debug": inst.get("debug", 0),
                            "engine": inst["engine"],
                            "ins": [], "outs": [],
                            "name": f"legw-{n}",
                            "opcode": "EventSemaphore",
                            "sync_info": {"on_update": [], "on_wait": [w_]},
                        })
                    si = dict(si)
                    si["on_wait"] = [waits[-1]]
                    inst = dict(inst)
                    inst["sync_info"] = si
                out.append(inst)
            bb["instructions"] = out
    return json.dumps(d).encode()


def _build(with_bias=True, kiter=KITER):
    nc = bass.Bass()
    # wb packs [wx(16x128) | wh(16x128) | ident(128) | wtag(2x20)] bf16 so
    # the whole weight set arrives in ONE DMA (each SW-DGE dma_start costs
    # ~680ns of Pool-queue issue time; 11 separate DMAs = ~7.5us of startup).
    WBC = 2 * 2048 + 128 + KC * TAGS
    xsT_e = nc.declare_dram_parameter("xsT", [E, BC * S], BF, isOutput=False)
    wb_e = nc.declare_dram_parameter("wb", [128, WBC], BF, isOutput=False)
    bvbt_e = nc.declare_dram_parameter("bvbt", [128, 9], F32, isOutput=False)
    out_e = nc.declare_dram_parameter("outT", [TAGS, BC * S], F32, isOutput=True)

    with tile.TileContext(nc) as tc:
        with (
            tc.tile_pool(name="big", bufs=1) as big,
            tc.tile_pool(name="ep", bufs=3) as ep,
            tc.tile_pool(name="gp", bufs=1, space="PSUM") as gp,
        ):
            # persistent SBUF (free dims are [slot/kc, b, t], batch-major)
            xs = big.tile([128, KC, BC * S], BF)
            xp = big.tile([128, 8, BC, S], BF)
            # h' history: [p, kc, b, 1+t]; col 0 = h'_{-1} = 0. In-place
            # across Picard sweeps.
            hst = big.tile([128, KC, BC, S + 1], BF)
            wb = big.tile([128, WBC], BF)
            bvbt = big.tile([128, 9], F32)
            outb = big.tile([TAGS, BC * S], F32)
            warm = big.tile([128, 8], F32)
            c05 = big.tile([128, 2, S], F32)  # constant 0.5 for the Pool ops

            def wx_sl(slot, kc):
                o = (slot * 2 + kc) * 128
                return wb[:, o:o + 128]

            def wh_sl(slot, kc):
                o = 2048 + (slot * 2 + kc) * 128
                return wb[:, o:o + 128]

            ident = wb[:, 4096:4224]

            def wt_sl(kc):
                o = 4224 + kc * TAGS
                return wb[:, o:o + TAGS]

            bv = bvbt[:, 0:8]
            bt = bvbt[0:TAGS, 8:9]

            # ---- input DMAs, ordered so iter-0's first GEMM unblocks
            # earliest: wx region of the bundle, then xs, then the rest ----
            nc.gpsimd.dma_start(wb[:, 0:2048], wb_e[:, 0:2048])
            nc.gpsimd.dma_start(xs[:, 0, :], xsT_e[0:128, :])
            nc.gpsimd.dma_start(xs[:, 1, :], xsT_e[128:256, :])
            nc.gpsimd.dma_start(wb[:, 2048:WBC], wb_e[:, 2048:WBC])
            nc.gpsimd.dma_start(bvbt[:], bvbt_e[:])

            nc.vector.memset(hst[:, :, :, 0:1], 0.0)
            nc.gpsimd.memset(c05[:], 0.5)
            # warm the sigmoid table before the pipeline needs it
            nc.vector.memset(warm[:], 0.0)
            nc.scalar.activation(warm[:], warm[:], AF.Sigmoid)

            ncnt = [0]

            def jtile(k, j):
                """One Picard-sweep stage for jtile j (batch rows 2j, 2j+1)."""
                # sall/s2 MUST be f32: u and h' subtract 0.5 from a sigmoid
                # output, and bf16's ~2e-3-absolute step near 0.5 becomes a
                # large relative error on small tanh values (feeds back
                # through the fixed point).  bf16 also HALVES the DVE scan
                # rate here (measured 1137ns vs 672ns per 256-col scan), so
                # f32 everywhere is also the faster choice.
                sall = ep.tile([128, 8, 2, S], F32, tag="sall")
                u = ep.tile([128, KC, 2 * S], F32, tag="u")
                cs = ep.tile([128, KC, 2, S], F32, tag="cs")
                for half in range(2):
                    off = 4 * half
                    ps = gp.tile([128, 4, 2, S], F32, tag=f"g{half}")
                    if k == 0:
                        # gates = xs @ Wx (+b); also materialize xp for later
                        for s4 in range(4):
                            for kc in range(KC):
                                nc.tensor.matmul(
                                    ps[:, s4, :, :],
                                    lhsT=wx_sl(off + s4, kc),
                                    rhs=xs[:, kc, 2 * j * S:(2 * j + 2) * S],
                                    start=(kc == 0), stop=(kc == KC - 1),
                                )
                        dst = xp[:, off:off + 4, 2 * j:2 * j + 2, :]
                        sdst = sall[:, off:off + 4, :, :]
                        if with_bias:
                            nc.vector.tensor_add(
                                dst, ps[:],
                                bv[:, off:off + 4].broadcast_to([128, 4, 2, S]))
                            nc.scalar.activation(sdst, dst, AF.Sigmoid)
                        else:
                            nc.scalar.activation(sdst, ps[:], AF.Sigmoid)
                            # split the psum->xp cast across DVE and ACT so
                            # the two halves copy in parallel
                            nc.vector.tensor_copy(
                                xp[:, off:off + 2, 2 * j:2 * j + 2, :],
                                ps[:, 0:2, :, :])
                            nc.scalar.copy(
                                xp[:, off + 2:off + 4, 2 * j:2 * j + 2, :],
                                ps[:, 2:4, :, :])
                    else:
                        # gates = xp + Wh' @ h'_{t-1}
                        for s4 in range(4):
                            # one per slot: a matmul's PSUM output must stay
                            # within a single 2KB bank (512 fp32)
                            nc.tensor.matmul(
                                ps[:, s4, :, :],
                                lhsT=ident,
                                rhs=xp[:, off + s4, 2 * j:2 * j + 2, :],
                                start=True, stop=False, skip_group_check=True,
                            )
                        for s4 in range(4):
                            for kc in range(KC):
                                nc.tensor.matmul(
                                    ps[:, s4, :, :],
                                    lhsT=wh_sl(off + s4, kc),
                                    rhs=hst[:, kc, 2 * j:2 * j + 2, 0:S],
                                    start=False,
                                    stop=(s4 == 3 and kc == KC - 1),
                                    skip_group_check=True,
                                )
                        nc.scalar.activation(
                            sall[:, off:off + 4, :, :], ps[:], AF.Sigmoid)
                # u = (sg2 - 0.5) * si = i*g/2   (one STT; output must be <=3D)
                nc.vector.scalar_tensor_tensor(
                    u[:], sall[:, 0:2, :, :], 0.5, sall[:, 2:4, :, :],
                    OP.subtract, OP.mult)
                # c' scan: c'_t = sf_t * c'_{t-1} + u_t  (c' = c/2)
                for kc in range(KC):
                    for b2 in range(2):
                        nc.vector.tensor_tensor_scan(
                            cs[:, kc, b2, :], sall[:, 4 + kc, b2, :],
                            u[:, kc, b2 * S:(b2 + 1) * S], 0.0, OP.mult, OP.add)
                return (j, sall, cs)

            def tail(pend):
                """Deferred s2/h' for a finished jtile — emitted one jtile
                later so the ACT queue never stalls on the DVE scans."""
                j, sall, cs = pend
                s2 = ep.tile([128, KC, 2, S], F32, tag="s2")
                # s2 = sigmoid(4*c') = sigmoid(2c)
                nc.scalar.activation(s2[:], cs[:], AF.Sigmoid, scale=4.0)
                # h' = (s2 - 0.5) * so = h/2   (STT output must be <=3D).
                # Keep these on DVE: offloading one to Pool (its TTs run
                # ~1.4us each) delays hst, which gates the NEXT sweep's GEMM.
                for kc in range(KC):
                    nc.vector.scalar_tensor_tensor(
                        hst[:, kc, 2 * j:2 * j + 2, 1:S + 1], s2[:, kc, :, :],
                        0.5, sall[:, 6 + kc, :, :], OP.subtract, OP.mult)

            pend = None
            for k in range(kiter):
                for j in range(NJT):
                    pend_new = jtile(k, j)
                    if pend is not None:
                        tail(pend)
                    pend = pend_new
            tail(pend)

            # ---- tag projection: chunks of 2 batch rows (512 cols) ----
            for j in range(NJT):
                pt = gp.tile([128, 4, 2, S], F32, tag="g0")
                for kc in range(KC):
                    nc.tensor.matmul(
                        pt[0:TAGS, 0, :, :],
                        lhsT=wt_sl(kc),
                        rhs=hst[:, kc, 2 * j:2 * j + 2, 1:S + 1],
                        start=(kc == 0), stop=(kc == KC - 1),
                    )
                for b2 in range(2):
                    dst = outb[:, (2 * j + b2) * S:(2 * j + b2 + 1) * S]
                    src = pt[0:TAGS, 0, b2, :]
                    if with_bias:
                        nc.vector.tensor_add(
                            dst, src, bt[:, 0:1].broadcast_to([TAGS, S]))
                    elif (j + b2) % 2 == 0:
                        nc.vector.tensor_copy(dst, src)
                    else:
                        nc.scalar.copy(dst, src)
            for h in range(2):
                HW_ = BC * S // 2
                nc.gpsimd.dma_start(out_e[:, h * HW_:(h + 1) * HW_],
                                    outb[:, h * HW_:(h + 1) * HW_])
    return nc


def _prep_w(Wmat, hscale):
    """[256, 1024] -> [128 part, slot 8, kc 2, m 128] bf16, slot-permuted.
    g-gate slots (0,1) scaled x2 (tanh via sigmoid); everything scaled by
    hscale (2 for Wh, which consumes h' = h/2)."""
    t = Wmat.reshape(KC, 128, 8, 128)[:, :, PERM, :].astype(np.float32).copy()
    t[:, :, 0:2, :] *= 2.0
    t *= hscale
    return np.ascontiguousarray(t.transpose(1, 2, 0, 3)).astype(BF16)


def _prep_b(b):
    """[1024] -> [128, 8] f32, slot-permuted per-partition bias (g x2)."""
    b8 = b.reshape(8, 128)[PERM, :].astype(np.float32).copy()
    b8[0:2, :] *= 2.0
    return np.ascontiguousarray(b8.T)


def kernel(x, emb, Wx_f, Wh_f, b_f, Wx_b, Wh_b, b_b, W_tag, b_tag):
    x = np.asarray(x)
    emb = np.asarray(emb, np.float32)
    Wx_f, Wh_f, b_f = (np.asarray(a, np.float32) for a in (Wx_f, Wh_f, b_f))
    Wx_b, Wh_b, b_b = (np.asarray(a, np.float32) for a in (Wx_b, Wh_b, b_b))
    W_tag = np.asarray(W_tag, np.float32)
    b_tag = np.asarray(b_tag, np.float32)

    with_bias = bool(b_f.any() or b_b.any() or b_tag.any())
    key = ("nc3", with_bias, KITER)
    if key not in _CACHE:
        nc = _build(with_bias=with_bias, kiter=KITER)
        legalized = _legalize_bir_waits(nc.to_json_bytes())
        nc.to_json_bytes = lambda: legalized  # shadow: feed legalized BIR to compile
        _CACHE[key] = nc
    nc = _CACHE[key]

    embeds = emb[x]  # [B, S, E] f32
    in_maps = []
    for core in range(8):
        fwd = core < 4
        c = core % 4
        eb = embeds[c * BC:(c + 1) * BC]  # [BC, S, E]
        if not fwd:
            eb = eb[:, ::-1, :]
        # batch-major tokens: col j = b*S + t
        xsT = np.ascontiguousarray(
            eb.transpose(2, 0, 1).reshape(E, BC * S)).astype(BF16)
        Wx, Wh, bb = (Wx_f, Wh_f, b_f) if fwd else (Wx_b, Wh_b, b_b)
        wth = (W_tag[:H2] if fwd else W_tag[H2:]) * 2.0  # x2: h' = h/2
        wt_d = np.ascontiguousarray(
            wth.reshape(KC, 128, TAGS).transpose(1, 0, 2)).astype(BF16)
        bt_d = (b_tag if fwd else np.zeros_like(b_tag)).astype(np.float32)
        # pack [wx | wh | ident | wtag] into one bf16 blob (single DMA)
        wb = np.concatenate([
            _prep_w(Wx, 1.0).reshape(128, 2048),
            _prep_w(Wh, 2.0).reshape(128, 2048),
            np.eye(128, dtype=BF16),
            wt_d.reshape(128, KC * TAGS),
        ], axis=1)
        bvbt = np.zeros((128, 9), np.float32)
        bvbt[:, 0:8] = _prep_b(bb)
        bvbt[0:TAGS, 8] = bt_d
        in_maps.append({
            "xsT": xsT,
            "wb": np.ascontiguousarray(wb),
            "bvbt": bvbt,
        })

    trace = bool(os.environ.get("BILSTM_TRACE"))
    global LAST_RESULT
    kw = {}
    if trace:
        import shutil
        kw["tmpdir"] = os.environ.get("BILSTM_TRACE_DIR", "/tmp/bilstm_trace")
        shutil.rmtree(kw["tmpdir"], ignore_errors=True)
        os.makedirs(kw["tmpdir"], exist_ok=True)
    res = run_bass_kernel_spmd(nc, in_maps, core_ids=list(range(8)), trace=trace, **kw)
    LAST_RESULT = res

    outs = [np.asarray(res.results[i]["outT"], np.float32).reshape(TAGS, BC, S)
            for i in range(8)]
    out = np.empty((B, S, TAGS), np.float32)
    for c in range(4):
        tot = outs[c] + outs[c + 4][:, :, ::-1]
        out[c * BC:(c + 1) * BC] = tot.transpose(1, 2, 0)
    return out
